# revision 21
# baseline (speedup 1.0000x reference)
"""Trainium2 Bass kernel for the CaputoEncoder model.

Model (see reference): feats = concat([caputo(x, 0.5), caputo(x, 1.0)], -1)
-> 2-layer LSTM(512) -> last timestep -> relu(linear).

Key simplifications:
  * caputo(x, 1.0) has coefficient 1/gamma(0) == 0 -> contributes zeros;
    only the alpha=0.5 branch matters, so only Wih0[:, :250] is ever used.
  * caputo(x, .5) = d*x - Wc@x (over time) == G @ x_b with G = diag(d) - Wc,
    host-precomputed; becomes a single matmul per batch.

Sharding: pure data parallelism over batch (64 -> 8 per core, 8 cores).
All weights replicated; scatter/gather on host.

On-core layout (hidden-major):
  hT, cT  : (128 part = hidden%128, cols = kchunk*8 + b)   [4*8=32 cols]
  gatesT  : (128 part = gate%128,  cols = gchunk*8 + b)    [16*8=128 cols]
  gate chunks host-permuted to [i, f, o, g] so sigmoid covers cols 0..95.
"""

import math
from contextlib import ExitStack

import numpy as np
import ml_dtypes

import concourse.bass as bass
import concourse.tile as tile
from concourse import mybir
from concourse.bass import ds
from concourse.bass_utils import run_bass_kernel_spmd

AF = mybir.ActivationFunctionType
OP = mybir.AluOpType
F32 = mybir.dt.float32
F32R = mybir.dt.float32r
BF16 = mybir.dt.bfloat16
F8 = mybir.dt.float8e4

B, T, N = 64, 512, 250
NP = 256          # n padded to 2 partition chunks
H = 512
G4 = 4 * H        # 2048
OUT = 1024
NCORES = 8
PB = B // NCORES  # 8 batches per core
WIN = 32          # scan steps per For_i iteration
NWIN = T // WIN

KC = H // 128     # 4 hidden chunks
GC = G4 // 128    # 16 gate chunks
NC2 = NP // 128   # 2 input chunks
CB = KC * PB      # 32 h/c columns
NS = 4            # independent scan streams (batch groups) per core
SB = PB // NS     # batches per stream
CSB = KC * SB     # h/c columns per stream


def _split_drain_waits(nc, max_waits=1):
    """This walrus build's CoreV3 codegen accepts at most one sem-wait per
    engine instruction (Drain/Matmult/... ISA structs have a single wait
    slot).  Move extra waits onto same-engine NoOps inserted immediately
    before the instruction — the engine blocks at the NoOp instead, which is
    semantically identical (same engine stream, same program point)."""
    for bb in nc.m.functions[0].blocks:
        insts = bb.instructions  # live list
        i = 0
        while i < len(insts):
            ins = insts[i]
            si = ins.sync_info
            if si is not None and len(si.on_wait) > max_waits:
                waits = list(si.on_wait)
                ins.sync_info = mybir.SyncInfo(
                    on_wait=waits[:max_waits], on_update=list(si.on_update)
                )
                for j, w in enumerate(waits[max_waits:]):
                    nop = mybir.InstNoOp(name=f"{ins.name}-wsplit{j}")
                    nop.engine = ins.engine
                    nop.sync_info = mybir.SyncInfo(on_wait=[w], on_update=[])
                    insts.insert(i, nop)
                    i += 1
            i += 1


def _scan_phase(nc, tc, whh_sb, xw_dram, hseq_dram, h_curs, c_curs, name):
    """One LSTM layer recurrence over T steps, as NS independent batch
    streams so one stream's elementwise chain overlaps another stream's
    matmul burst (and the PE stays busy enough to hold the warm clock).

    whh_sb   : SBUF (128, KC, G4) fp8e4  (Whh.T chunks, gates permuted [i,f,o,g])
    xw_dram  : DRAM (GC, 128, T, PB) f32  precomputed input contribution + bias
    hseq_dram: DRAM (NWIN, 128, KC*WIN*PB) bf16 or None; h_t sequence dumped
               one window at a time (cols = kc*WIN*PB + u*PB + b)
    h_curs/c_curs: per-stream persistent (128, CSB) bf16 / f32 tiles, pre-zeroed
    """

    with ExitStack() as sctx:
        win_pool = sctx.enter_context(tc.tile_pool(name=f"{name}_win", bufs=2))
        ps_pools = [
            sctx.enter_context(
                tc.tile_pool(name=f"{name}_ps{s}", bufs=2, space="PSUM")
            )
            for s in range(NS)
        ]
        ew_pool = sctx.enter_context(tc.tile_pool(name=f"{name}_ew", bufs=3))
        hw_pool = sctx.enter_context(tc.tile_pool(name=f"{name}_hw", bufs=3))
        with tc.For_i(0, NWIN, 1, hint_engines=(mybir.EngineType.PE,)) as iw:
            # prefetch xw for WIN steps: cols = gc*(WIN*PB) + t*PB + b
            win = win_pool.tile([128, GC * WIN * PB], F32, tag="win")
            nc.sync.dma_start(
                win[:].rearrange("p (g w b) -> p g w b", g=GC, w=WIN),
                xw_dram[:, :, ds(iw * WIN, WIN), :].rearrange(
                    "g p w b -> p g w b"
                ),
            )
            # per-step view with dims (p, g, b)
            win_v = win[:].rearrange("p (g w b) -> p w g b", g=GC, w=WIN)
            # hwin layout: cols = kc*(WIN*PB) + u*PB + b  (kc-major for the
            # per-kc window stores, which must be 2D-contiguous)
            hwin = None
            if hseq_dram is not None:
                hwin = win_pool.tile([128, KC * WIN * PB], BF16, tag="hwin")
                hwin_v = hwin.rearrange("p (k w b) -> p w k b", k=KC, w=WIN)
            # per-stream h as KC APs (128, SB)
            h_prev = [
                [h_curs[s][:, kc * SB:(kc + 1) * SB] for kc in range(KC)]
                for s in range(NS)
            ]
            c_prev = list(c_curs)
            for u in range(WIN):
                for s in range(NS):
                    b0 = s * SB
                    psum = ps_pools[s].tile([128, GC * SB], F32, tag="ps")
                    for gc in range(GC):
                        for kc in range(KC):
                            nc.tensor.matmul(
                                psum[:, gc * SB:(gc + 1) * SB],
                                whh_sb[:, kc, gc * 128:(gc + 1) * 128],
                                h_prev[s][kc],
                                start=(kc == 0),
                                stop=(kc == KC - 1),
                            )
                    # gates = psum + xw[t]
                    gates = ew_pool.tile([128, GC * SB], F32, tag=f"gates{s}")
                    nc.vector.scalar_tensor_tensor(
                        gates.rearrange("p (g b) -> p g b", g=GC),
                        psum.rearrange("p (g b) -> p g b", g=GC),
                        1.0,
                        win_v[:, u, :, b0:b0 + SB],
                        OP.mult,
                        OP.add,
                    )
                    # i,f,o sigmoid on cols [0, 3*CSB); g tanh on the rest
                    acts = ew_pool.tile([128, GC * SB], F32, tag=f"acts{s}")
                    nc.scalar.activation(
                        acts[:, :3 * CSB], gates[:, :3 * CSB], AF.Sigmoid
                    )
                    nc.scalar.activation(
                        acts[:, 3 * CSB:], gates[:, 3 * CSB:], AF.Tanh
                    )
                    # c = f*c + i*g ; h = o*tanh(c)
                    ig = ew_pool.tile([128, CSB], F32, tag=f"ig{s}")
                    nc.vector.tensor_tensor(
                        ig[:], acts[:, :CSB], acts[:, 3 * CSB:], OP.mult
                    )
                    fc = ew_pool.tile([128, CSB], F32, tag=f"fc{s}")
                    nc.vector.tensor_tensor(
                        fc[:], acts[:, CSB:2 * CSB], c_prev[s][:], OP.mult
                    )
                    c_new = (
                        c_curs[s]
                        if u == WIN - 1
                        else hw_pool.tile([128, CSB], F32, tag=f"c{s}")
                    )
                    nc.vector.tensor_tensor(c_new[:], fc[:], ig[:], OP.add)
                    tc_t = ew_pool.tile([128, CSB], F32, tag=f"tc{s}")
                    nc.scalar.activation(tc_t[:], c_new[:], AF.Tanh)
                    acts_o = acts[:, 2 * CSB:3 * CSB].rearrange(
                        "p (k b) -> p k b", k=KC
                    )
                    tc_v = tc_t[:].rearrange("p (k b) -> p k b", k=KC)
                    if hwin is not None:
                        h_out = hwin_v[:, u, :, b0:b0 + SB]
                    elif u == WIN - 1:
                        h_out = h_curs[s][:].rearrange("p (k b) -> p k b", k=KC)
                    else:
                        h_tmp = hw_pool.tile([128, CSB], BF16, tag=f"h{s}")
                        h_out = h_tmp[:].rearrange("p (k b) -> p k b", k=KC)
                    nc.vector.tensor_tensor(h_out, acts_o, tc_v, OP.mult)
                    if hwin is not None and u == WIN - 1:
                        nc.vector.tensor_copy(
                            h_curs[s][:].rearrange("p (k b) -> p k b", k=KC),
                            h_out,
                        )
                    h_prev[s] = [h_out[:, kc, :] for kc in range(KC)]
                    c_prev[s] = c_new
            if hwin is not None:
                nc.sync.dma_start(hseq_dram[ds(iw, 1), :, :], hwin[:])


def build_nc():
    nc = bass.Bass()

    x_in = nc.dram_tensor("x", [PB, T, NP], F32, kind="ExternalInput")
    gt_in = nc.dram_tensor("gt", [KC, 128, T], F32, kind="ExternalInput")
    a0_in = nc.dram_tensor("a0t", [NC2, 128, G4], BF16, kind="ExternalInput")
    b0_in = nc.dram_tensor("b0", [128, GC], F32, kind="ExternalInput")
    whh0_in = nc.dram_tensor("whh0t", [KC, 128, G4], BF16, kind="ExternalInput")
    a1_in = nc.dram_tensor("a1t", [KC, 128, G4], BF16, kind="ExternalInput")
    b1_in = nc.dram_tensor("b1", [128, GC], F32, kind="ExternalInput")
    whh1_in = nc.dram_tensor("whh1t", [KC, 128, G4], BF16, kind="ExternalInput")
    wout_in = nc.dram_tensor("woutt", [KC, 128, OUT], BF16, kind="ExternalInput")
    bout_in = nc.dram_tensor("boutr", [PB, OUT], F32, kind="ExternalInput")
    out_ext = nc.dram_tensor("out", [PB, OUT], F32, kind="ExternalOutput")

    xw0_dram = nc.dram_tensor("xw0s", [GC, 128, T, PB], F32)
    xw1_dram = nc.dram_tensor("xw1s", [GC, 128, T, PB], F32)
    h0seq_dram = nc.dram_tensor("h0seqs", [NWIN, 128, KC * WIN * PB], BF16)

    with tile.TileContext(nc) as tc:
        with ExitStack() as ctx:
            const_pool = ctx.enter_context(tc.tile_pool(name="consts", bufs=1))
            state_pool = ctx.enter_context(tc.tile_pool(name="state", bufs=1))

            gt_sb = const_pool.tile([128, KC, T], F32)
            nc.sync.dma_start(gt_sb[:], gt_in[:, :, :].rearrange("k p t -> p k t"))
            a0_sb = const_pool.tile([128, NC2, G4], BF16)
            nc.sync.dma_start(a0_sb[:], a0_in[:, :, :].rearrange("k p g -> p k g"))
            b0_sb = const_pool.tile([128, GC], F32)
            nc.sync.dma_start(b0_sb[:], b0_in[:, :])
            whh0_sb = const_pool.tile([128, KC, G4], BF16)
            nc.sync.dma_start(whh0_sb[:], whh0_in[:, :, :].rearrange("k p g -> p k g"))
            a1_sb = const_pool.tile([128, KC, G4], BF16)
            nc.sync.dma_start(a1_sb[:], a1_in[:, :, :].rearrange("k p g -> p k g"))
            b1_sb = const_pool.tile([128, GC], F32)
            nc.sync.dma_start(b1_sb[:], b1_in[:, :])
            whh1_sb = const_pool.tile([128, KC, G4], BF16)
            nc.sync.dma_start(whh1_sb[:], whh1_in[:, :, :].rearrange("k p g -> p k g"))
            wout_sb = const_pool.tile([128, KC, OUT], BF16)
            nc.sync.dma_start(wout_sb[:], wout_in[:, :, :].rearrange("k p g -> p k g"))
            bout_sb = const_pool.tile([PB, OUT], F32)
            nc.sync.dma_start(bout_sb[:], bout_in[:, :])

            # ---- phase A+B: featsT_b = x_bT @ G^T ; xw0 = A0 @ feats + b0 ----
            with tc.tile_pool(name="ab", bufs=2) as ab_pool, \
                 tc.tile_pool(name="abf", bufs=1) as abf_pool, \
                 tc.tile_pool(name="abps", bufs=2, space="PSUM") as abps_pool:
                feats = []
                for b in range(PB):
                    x_sb = ab_pool.tile([128, KC, NP], F32, tag="x")
                    nc.sync.dma_start(
                        x_sb[:], x_in[b].rearrange("(k p) n -> p k n", p=128)
                    )
                    fb = abf_pool.tile([128, NC2, T], BF16, tag=f"feats{b}")
                    for mc in range(NC2):
                        psA = abps_pool.tile([128, T], F32, tag="psA")
                        for kc in range(KC):
                            # f32r streams 1 col/cycle at free>=256 (f32: 4)
                            nc.tensor.matmul(
                                psA[:],
                                x_sb[:, kc, mc * 128:(mc + 1) * 128].bitcast(F32R),
                                gt_sb[:, kc, :].bitcast(F32R),
                                start=(kc == 0),
                                stop=(kc == KC - 1),
                            )
                        nc.vector.tensor_copy(fb[:, mc, :], psA[:])
                    feats.append(fb)
                for gc in range(GC):
                    xw_sb = ab_pool.tile([128, T * PB], F32, tag="xw")
                    xw_v = xw_sb[:].rearrange("p (t b) -> p b t", b=PB)
                    for b in range(PB):
                        psB = abps_pool.tile([128, T], F32, tag="psB")
                        for kc in range(NC2):
                            nc.tensor.matmul(
                                psB[:],
                                a0_sb[:, kc, gc * 128:(gc + 1) * 128],
                                feats[b][:, kc, :],
                                start=(kc == 0),
                                stop=(kc == NC2 - 1),
                            )
                        nc.scalar.activation(
                            xw_v[:, b, :], psB[:], AF.Identity,
                            bias=b0_sb[:, gc:gc + 1],
                        )
                    nc.sync.dma_start(
                        xw0_dram[gc].rearrange("p t b -> p (t b)"), xw_sb[:]
                    )

            # ---- phase C: layer-0 scan ----
            h0_curs = [
                state_pool.tile([128, CSB], BF16, tag=f"h0c{s}", name=f"h0c{s}")
                for s in range(NS)
            ]
            c0_curs = [
                state_pool.tile([128, CSB], F32, tag=f"c0c{s}", name=f"c0c{s}")
                for s in range(NS)
            ]
            for s in range(NS):
                nc.vector.memset(h0_curs[s][:], 0.0)
                nc.vector.memset(c0_curs[s][:], 0.0)
            _scan_phase(
                nc, tc, whh0_sb, xw0_dram, h0seq_dram, h0_curs, c0_curs, "s0"
            )

            # ---- phase D: xw1 = A1 @ h0seq + b1 ----
            with tc.tile_pool(name="d_rhs", bufs=1) as drhs_pool, \
                 tc.tile_pool(name="d_ps", bufs=2, space="PSUM") as dps_pool, \
                 tc.tile_pool(name="d_o", bufs=2) as do_pool:
                h0T = []
                for kc in range(KC):
                    t_ = drhs_pool.tile([128, T * PB], BF16, tag=f"h0_{kc}")
                    # gather this kc's (t, b) block from every window dump
                    nc.sync.dma_start(
                        t_[:].rearrange("p (w ub) -> p w ub", w=NWIN),
                        h0seq_dram[:, :, kc * WIN * PB:(kc + 1) * WIN * PB]
                        .rearrange("w p ub -> p w ub"),
                    )
                    h0T.append(t_[:].rearrange("p (t b) -> p b t", b=PB))
                for gc in range(GC):
                    xw_sb = do_pool.tile([128, T * PB], F32, tag="xw")
                    xw_v = xw_sb[:].rearrange("p (t b) -> p b t", b=PB)
                    for b in range(PB):
                        psum = dps_pool.tile([128, T], F32, tag="ps")
                        for kc in range(KC):
                            nc.tensor.matmul(
                                psum[:],
                                a1_sb[:, kc, gc * 128:(gc + 1) * 128],
                                h0T[kc][:, b, :],
                                start=(kc == 0),
                                stop=(kc == KC - 1),
                            )
                        nc.scalar.activation(
                            xw_v[:, b, :], psum[:], AF.Identity,
                            bias=b1_sb[:, gc:gc + 1],
                        )
                    nc.sync.dma_start(
                        xw1_dram[gc].rearrange("p t b -> p (t b)"), xw_sb[:]
                    )

            # ---- phase E: layer-1 scan ----
            h1_curs = [
                state_pool.tile([128, CSB], BF16, tag=f"h1c{s}", name=f"h1c{s}")
                for s in range(NS)
            ]
            c1_curs = [
                state_pool.tile([128, CSB], F32, tag=f"c1c{s}", name=f"c1c{s}")
                for s in range(NS)
            ]
            for s in range(NS):
                nc.vector.memset(h1_curs[s][:], 0.0)
                nc.vector.memset(c1_curs[s][:], 0.0)
            _scan_phase(nc, tc, whh1_sb, xw1_dram, None, h1_curs, c1_curs, "s1")
            # gather per-stream h1 into one (128, CB) tile, cols kc*PB + b
            h1_cur = state_pool.tile([128, CB], BF16)
            for s in range(NS):
                nc.vector.tensor_copy(
                    h1_cur[:].rearrange("p (k b) -> p k b", k=KC)[
                        :, :, s * SB:(s + 1) * SB
                    ],
                    h1_curs[s][:].rearrange("p (k b) -> p k b", k=KC),
                )

            # ---- phase F: out = relu(h1_last @ Wout.T + bout) ----
            with tc.tile_pool(name="f_ps", bufs=2, space="PSUM") as fps_pool, \
                 tc.tile_pool(name="f_o", bufs=1) as fo_pool:
                out_sb = fo_pool.tile([PB, OUT], F32)
                for half in range(2):
                    psF = fps_pool.tile([PB, 512], F32, tag="psF")
                    for kc in range(KC):
                        nc.tensor.matmul(
                            psF[:],
                            h1_cur[:, kc * PB:(kc + 1) * PB],
                            wout_sb[:, kc, half * 512:(half + 1) * 512],
                            start=(kc == 0),
                            stop=(kc == KC - 1),
                        )
                    sl = slice(half * 512, (half + 1) * 512)
                    nc.vector.tensor_tensor(
                        out_sb[:, sl], psF[:], bout_sb[:, sl], OP.add
                    )
                    nc.vector.tensor_scalar_max(out_sb[:, sl], out_sb[:, sl], 0.0)
                nc.sync.dma_start(out_ext[:, :], out_sb[:])

    _split_drain_waits(nc)
    return nc


_NC_CACHE = None


def _get_nc():
    global _NC_CACHE
    if _NC_CACHE is None:
        _NC_CACHE = build_nc()
    return _NC_CACHE


def _prep_host(inputs):
    x = np.asarray(inputs["x"], dtype=np.float32)
    coef = 1.0 / math.gamma(0.5)
    t = np.arange(T, dtype=np.float64)
    diff = t[:, None] - t[None, :]
    W = np.where(diff > 0, (np.abs(diff) + 1e-6) ** -0.5, 0.0).astype(np.float32)
    d = (coef * W.sum(1)).astype(np.float32)
    G = (np.diag(d) - coef * W).astype(np.float32)  # feats_b = G @ x_b
    GT = np.ascontiguousarray(G.T).reshape(KC, 128, T)

    perm = np.concatenate([  # torch gate order i,f,g,o -> [i,f,o,g]
        np.arange(0, H), np.arange(H, 2 * H),
        np.arange(3 * H, 4 * H), np.arange(2 * H, 3 * H),
    ])
    bf = ml_dtypes.bfloat16

    A0 = np.zeros((G4, NP), np.float32)
    A0[:, :N] = np.asarray(inputs["Wih0"], np.float32)[perm, :N]
    A0T = np.ascontiguousarray(A0.T).astype(bf).reshape(NC2, 128, G4)
    b0 = (np.asarray(inputs["bih0"], np.float32)
          + np.asarray(inputs["bhh0"], np.float32))[perm]
    b0_t = np.ascontiguousarray(b0.reshape(GC, 128).T)
    Whh0T = np.ascontiguousarray(
        np.asarray(inputs["Whh0"], np.float32)[perm].T
    ).astype(bf).reshape(KC, 128, G4)

    A1T = np.ascontiguousarray(
        np.asarray(inputs["Wih1"], np.float32)[perm].T
    ).astype(bf).reshape(KC, 128, G4)
    b1 = (np.asarray(inputs["bih1"], np.float32)
          + np.asarray(inputs["bhh1"], np.float32))[perm]
    b1_t = np.ascontiguousarray(b1.reshape(GC, 128).T)
    Whh1T = np.ascontiguousarray(
        np.asarray(inputs["Whh1"], np.float32)[perm].T
    ).astype(bf).reshape(KC, 128, G4)

    WoutT = np.ascontiguousarray(
        np.asarray(inputs["Wout"], np.float32).T
    ).astype(bf).reshape(KC, 128, OUT)
    bout_r = np.broadcast_to(
        np.asarray(inputs["bout"], np.float32), (PB, OUT)
    ).copy()

    xp = np.zeros((B, T, NP), np.float32)
    xp[:, :, :N] = x

    shared = dict(
        gt=GT, a0t=A0T, b0=b0_t, whh0t=Whh0T, a1t=A1T, b1=b1_t,
        whh1t=Whh1T, woutt=WoutT, boutr=bout_r,
    )
    in_maps = []
    for c in range(NCORES):
        m = dict(shared)
        m["x"] = np.ascontiguousarray(xp[c * PB:(c + 1) * PB])
        in_maps.append(m)
    return in_maps


def kernel(**inputs):
    nc = _get_nc()
    in_maps = _prep_host(inputs)
    res = run_bass_kernel_spmd(nc, in_maps, core_ids=list(range(NCORES)))
    out = np.concatenate([r["out"] for r in res.results], axis=0)
    return out.astype(np.float32)



# revision 31
# speedup vs baseline: 3.8142x; 3.8142x over previous
"""Trainium2 Bass kernel for the CaputoEncoder model.

Model (see reference): feats = concat([caputo(x, 0.5), caputo(x, 1.0)], -1)
-> 2-layer LSTM(512) -> last timestep -> relu(linear).

Key simplifications:
  * caputo(x, 1.0) has coefficient 1/gamma(0) == 0 -> contributes zeros;
    only the alpha=0.5 branch matters, so only Wih0[:, :250] is ever used.
  * caputo(x, .5) = d*x - Wc@x (over time) == G @ x_b with G = diag(d) - Wc,
    host-precomputed; becomes a single matmul per batch.

Sharding: pure data parallelism over batch (64 -> 8 per core, 8 cores).
All weights replicated; scatter/gather on host.

Layer pipelining: the two LSTM scans run interleaved step-by-step in one
hardware loop, with layer 1 one window (32 steps) behind layer 0.  While
layer 0's serial sigmoid/tanh/c/h chain runs on Act/DVE, layer 1's 64
weight-switching matmuls stream on the PE (and vice versa), instead of the
PE idling ~60% as in a single-layer scan.  The xw1 = A1 @ h0 + b1 input
transform for the next layer-1 window is computed in bulk (free dim 256)
from the SBUF-resident h0 window at each loop iteration's tail.

On-core layout (hidden-major):
  hT, cT  : (128 part = hidden%128, cols = kchunk*8 + b)   [4*8=32 cols]
  gatesT  : (128 part = gate%128,  cols = gchunk*8 + b)    [16*8=128 cols]
  gate chunks host-permuted to [i, f, o, g] so sigmoid covers cols 0..95.
"""

import math
from contextlib import ExitStack

import numpy as np
import ml_dtypes

import concourse.bass as bass
import concourse.tile as tile
from concourse import mybir
from concourse.bass import ds
from concourse.bass_utils import run_bass_kernel_spmd

AF = mybir.ActivationFunctionType
OP = mybir.AluOpType
F32 = mybir.dt.float32
F32R = mybir.dt.float32r
BF16 = mybir.dt.bfloat16

B, T, N = 64, 512, 250
NP = 256          # n padded to 2 partition chunks
H = 512
G4 = 4 * H        # 2048
OUT = 1024
NCORES = 8
PB = B // NCORES  # 8 batches per core
WIN = 32          # scan steps per For_i iteration
NWIN = T // WIN

KC = H // 128     # 4 hidden chunks
GC = G4 // 128    # 16 gate chunks
NC2 = NP // 128   # 2 input chunks
CB = KC * PB      # 32 h/c columns


def _split_drain_waits(nc, max_waits=1):
    """This walrus build's CoreV3 codegen accepts at most one sem-wait per
    engine instruction (Drain/Matmult/... ISA structs have a single wait
    slot).  Move extra waits onto same-engine NoOps inserted immediately
    before the instruction — the engine blocks at the NoOp instead, which is
    semantically identical (same engine stream, same program point)."""
    for bb in nc.m.functions[0].blocks:
        insts = bb.instructions  # live list
        i = 0
        while i < len(insts):
            ins = insts[i]
            si = ins.sync_info
            if si is not None and len(si.on_wait) > max_waits:
                waits = list(si.on_wait)
                ins.sync_info = mybir.SyncInfo(
                    on_wait=waits[:max_waits], on_update=list(si.on_update)
                )
                for j, w in enumerate(waits[max_waits:]):
                    nop = mybir.InstNoOp(name=f"{ins.name}-wsplit{j}")
                    nop.engine = ins.engine
                    nop.sync_info = mybir.SyncInfo(on_wait=[w], on_update=[])
                    insts.insert(i, nop)
                    i += 1
            i += 1


def _emit_step(nc, ps_pool, ew_pool, hw_pool, whh_sb, xw_u, st, u, tag):
    """Emit one LSTM step for one layer.

    xw_u : AP view (128, GC, PB) — precomputed input contribution + bias
    st   : dict with h_prev (list of KC APs (128, PB)), c_prev (AP),
           h_cur / c_cur (persistent tiles), hwin_v (or None) — where to
           write h, and its layout (p, w, k, b).
    """
    psum = ps_pool.tile([128, GC * PB], F32, tag=f"ps{tag}", name=f"ps{tag}")
    for gc in range(GC):
        for kc in range(KC):
            nc.tensor.matmul(
                psum[:, gc * PB:(gc + 1) * PB],
                whh_sb[:, kc, gc * 128:(gc + 1) * 128],
                st["h_prev"][kc],
                start=(kc == 0),
                stop=(kc == KC - 1),
            )
    gates = ew_pool.tile([128, GC * PB], F32, tag=f"g{tag}", name=f"g{tag}")
    nc.vector.scalar_tensor_tensor(
        gates.rearrange("p (g b) -> p g b", g=GC),
        psum.rearrange("p (g b) -> p g b", g=GC),
        1.0,
        xw_u,
        OP.mult,
        OP.add,
    )
    acts = ew_pool.tile([128, GC * PB], F32, tag=f"a{tag}", name=f"a{tag}")
    nc.scalar.activation(acts[:, :3 * CB], gates[:, :3 * CB], AF.Sigmoid)
    nc.scalar.activation(acts[:, 3 * CB:], gates[:, 3 * CB:], AF.Tanh)
    ig = ew_pool.tile([128, CB], F32, tag=f"ig{tag}", name=f"ig{tag}")
    nc.vector.tensor_tensor(ig[:], acts[:, :CB], acts[:, 3 * CB:], OP.mult)
    fc = ew_pool.tile([128, CB], F32, tag=f"fc{tag}", name=f"fc{tag}")
    nc.vector.tensor_tensor(fc[:], acts[:, CB:2 * CB], st["c_prev"][:], OP.mult)
    c_new = (
        st["c_cur"]
        if u == WIN - 1
        else hw_pool.tile([128, CB], F32, tag=f"c{tag}", name=f"c{tag}")
    )
    nc.vector.tensor_tensor(c_new[:], fc[:], ig[:], OP.add)
    tc_t = ew_pool.tile([128, CB], F32, tag=f"tc{tag}", name=f"tc{tag}")
    nc.scalar.activation(tc_t[:], c_new[:], AF.Tanh)
    acts_o = acts[:, 2 * CB:3 * CB].rearrange("p (k b) -> p k b", k=KC)
    tc_v = tc_t[:].rearrange("p (k b) -> p k b", k=KC)
    if st["hwin_v"] is not None:
        h_out = st["hwin_v"][:, u]
    elif u == WIN - 1:
        h_out = st["h_cur"][:].rearrange("p (k b) -> p k b", k=KC)
    else:
        h_tmp = hw_pool.tile([128, CB], BF16, tag=f"h{tag}", name=f"h{tag}")
        h_out = h_tmp[:].rearrange("p (k b) -> p k b", k=KC)
    nc.vector.tensor_tensor(h_out, acts_o, tc_v, OP.mult)
    if st["hwin_v"] is not None and u == WIN - 1:
        nc.vector.tensor_copy(
            st["h_cur"][:].rearrange("p (k b) -> p k b", k=KC), h_out
        )
    st["h_prev"] = [h_out[:, kc, :] for kc in range(KC)]
    st["c_prev"] = c_new


def _emit_bulk_xw1(nc, bps_pool, bxw_pool, a1_sb, b1_sb, hwin, xw1_dram, wi):
    """xw1_dram[wi] = A1 @ h0(window) + b1, from the SBUF-resident h0
    window (128, KC*WIN*PB) bf16.  free dim = WIN*PB = 256.
    xw1_dram: (NWIN, 128, GC*WIN*PB), cols = gc*(WIN*PB) + w*PB + b.
    wi may be an int or a loop-index expression."""
    hv = hwin.rearrange("p (k wb) -> p k wb", k=KC)
    stg = bxw_pool.tile([128, GC * WIN * PB], BF16, tag="bxw", name="bxw")
    for gc in range(GC):
        psum = bps_pool.tile([128, WIN * PB], F32, tag="bps", name="bps")
        for kc in range(KC):
            nc.tensor.matmul(
                psum[:],
                a1_sb[:, kc, gc * 128:(gc + 1) * 128],
                hv[:, kc],
                start=(kc == 0),
                stop=(kc == KC - 1),
            )
        nc.scalar.activation(
            stg[:, gc * WIN * PB:(gc + 1) * WIN * PB], psum[:],
            AF.Identity, bias=b1_sb[:, gc:gc + 1],
        )
    nc.sync.dma_start(xw1_dram[ds(wi, 1), :, :], stg[:])


def build_nc():
    nc = bass.Bass()

    x_in = nc.dram_tensor("x", [PB, T, NP], F32, kind="ExternalInput")
    gt_in = nc.dram_tensor("gt", [KC, 128, T], F32, kind="ExternalInput")
    a0_in = nc.dram_tensor("a0t", [NC2, 128, G4], BF16, kind="ExternalInput")
    b0_in = nc.dram_tensor("b0", [128, GC], F32, kind="ExternalInput")
    whh0_in = nc.dram_tensor("whh0t", [KC, 128, G4], BF16, kind="ExternalInput")
    a1_in = nc.dram_tensor("a1t", [KC, 128, G4], BF16, kind="ExternalInput")
    b1_in = nc.dram_tensor("b1", [128, GC], F32, kind="ExternalInput")
    whh1_in = nc.dram_tensor("whh1t", [KC, 128, G4], BF16, kind="ExternalInput")
    wout_in = nc.dram_tensor("woutt", [KC, 128, OUT], BF16, kind="ExternalInput")
    bout_in = nc.dram_tensor("boutr", [PB, OUT], F32, kind="ExternalInput")
    out_ext = nc.dram_tensor("out", [PB, OUT], F32, kind="ExternalOutput")

    xw0_dram = nc.dram_tensor("xw0s", [GC, 128, T, PB], F32)
    xw1_dram = nc.dram_tensor("xw1s", [NWIN, 128, GC * WIN * PB], BF16)

    with tile.TileContext(nc) as tc:
        with ExitStack() as ctx:
            const_pool = ctx.enter_context(tc.tile_pool(name="consts", bufs=1))
            state_pool = ctx.enter_context(tc.tile_pool(name="state", bufs=1))

            gt_sb = const_pool.tile([128, KC, T], F32)
            nc.sync.dma_start(gt_sb[:], gt_in[:, :, :].rearrange("k p t -> p k t"))
            a0_sb = const_pool.tile([128, NC2, G4], BF16)
            nc.sync.dma_start(a0_sb[:], a0_in[:, :, :].rearrange("k p g -> p k g"))
            b0_sb = const_pool.tile([128, GC], F32)
            nc.sync.dma_start(b0_sb[:], b0_in[:, :])
            whh0_sb = const_pool.tile([128, KC, G4], BF16)
            nc.sync.dma_start(whh0_sb[:], whh0_in[:, :, :].rearrange("k p g -> p k g"))
            a1_sb = const_pool.tile([128, KC, G4], BF16)
            nc.sync.dma_start(a1_sb[:], a1_in[:, :, :].rearrange("k p g -> p k g"))
            b1_sb = const_pool.tile([128, GC], F32)
            nc.sync.dma_start(b1_sb[:], b1_in[:, :])
            whh1_sb = const_pool.tile([128, KC, G4], BF16)
            nc.sync.dma_start(whh1_sb[:], whh1_in[:, :, :].rearrange("k p g -> p k g"))
            wout_sb = const_pool.tile([128, KC, OUT], BF16)
            nc.sync.dma_start(wout_sb[:], wout_in[:, :, :].rearrange("k p g -> p k g"))
            bout_sb = const_pool.tile([PB, OUT], F32)
            nc.sync.dma_start(bout_sb[:], bout_in[:, :])

            # ---- phase A+B: featsT_b = x_bT @ G^T ; xw0 = A0 @ feats + b0 ----
            with tc.tile_pool(name="ab", bufs=2) as ab_pool, \
                 tc.tile_pool(name="abf", bufs=1) as abf_pool, \
                 tc.tile_pool(name="abps", bufs=2, space="PSUM") as abps_pool:
                feats = []
                for b in range(PB):
                    x_sb = ab_pool.tile([128, KC, NP], F32, tag="x")
                    nc.sync.dma_start(
                        x_sb[:], x_in[b].rearrange("(k p) n -> p k n", p=128)
                    )
                    fb = abf_pool.tile([128, NC2, T], BF16, tag=f"feats{b}")
                    for mc in range(NC2):
                        psA = abps_pool.tile([128, T], F32, tag="psA")
                        for kc in range(KC):
                            nc.tensor.matmul(
                                psA[:],
                                x_sb[:, kc, mc * 128:(mc + 1) * 128],
                                gt_sb[:, kc, :],
                                start=(kc == 0),
                                stop=(kc == KC - 1),
                            )
                        nc.vector.tensor_copy(fb[:, mc, :], psA[:])
                    feats.append(fb)
                for gc in range(GC):
                    xw_sb = ab_pool.tile([128, T * PB], F32, tag="xw")
                    xw_v = xw_sb[:].rearrange("p (t b) -> p b t", b=PB)
                    for b in range(PB):
                        psB = abps_pool.tile([128, T], F32, tag="psB")
                        for kc in range(NC2):
                            nc.tensor.matmul(
                                psB[:],
                                a0_sb[:, kc, gc * 128:(gc + 1) * 128],
                                feats[b][:, kc, :],
                                start=(kc == 0),
                                stop=(kc == NC2 - 1),
                            )
                        nc.scalar.activation(
                            xw_v[:, b, :], psB[:], AF.Identity,
                            bias=b0_sb[:, gc:gc + 1],
                        )
                    nc.sync.dma_start(
                        xw0_dram[gc].rearrange("p t b -> p (t b)"), xw_sb[:]
                    )

            # ---- merged scans: L0 windows 0..15, L1 lagging one window ----
            h0_cur = state_pool.tile([128, CB], BF16)
            c0_cur = state_pool.tile([128, CB], F32)
            h1_cur = state_pool.tile([128, CB], BF16)
            c1_cur = state_pool.tile([128, CB], F32)
            for t_ in (c0_cur, c1_cur):
                nc.vector.memset(t_[:], 0.0)
            for t_ in (h0_cur, h1_cur):
                nc.vector.memset(t_[:], 0.0)

            st0 = {"h_cur": h0_cur, "c_cur": c0_cur}
            st1 = {"h_cur": h1_cur, "c_cur": c1_cur, "hwin_v": None}

            with ExitStack() as sctx:
                win_pool = sctx.enter_context(tc.tile_pool(name="win", bufs=2))
                ps0_pool = sctx.enter_context(
                    tc.tile_pool(name="ps0", bufs=2, space="PSUM"))
                ps1_pool = sctx.enter_context(
                    tc.tile_pool(name="ps1", bufs=2, space="PSUM"))
                bps_pool = sctx.enter_context(
                    tc.tile_pool(name="bps", bufs=2, space="PSUM"))
                bxw_pool = sctx.enter_context(tc.tile_pool(name="bxw", bufs=2))
                ew_pool = sctx.enter_context(tc.tile_pool(name="ew", bufs=3))
                hw_pool = sctx.enter_context(tc.tile_pool(name="hw", bufs=3))

                def l0_window(woff):
                    """Emit L0's scan for xw0 window at woff; returns nothing.
                    Interleaved with l1 steps by the caller via gen."""
                    win0 = win_pool.tile([128, GC * WIN * PB], F32, tag="w0",
                                         name="w0")
                    nc.sync.dma_start(
                        win0[:].rearrange("p (g w b) -> p g w b", g=GC, w=WIN),
                        xw0_dram[:, :, ds(woff, WIN), :].rearrange(
                            "g p w b -> p g w b"),
                    )
                    hwin = win_pool.tile([128, KC * WIN * PB], BF16, tag="hw0",
                                         name="hw0")
                    st0["hwin_v"] = hwin.rearrange(
                        "p (k w b) -> p w k b", k=KC, w=WIN)
                    win0_v = win0[:].rearrange(
                        "p (g w b) -> p w g b", g=GC, w=WIN)
                    st0["h_prev"] = [
                        h0_cur[:, kc * PB:(kc + 1) * PB] for kc in range(KC)]
                    st0["c_prev"] = c0_cur
                    return win0_v, hwin

                def l1_window(wi):
                    win1 = win_pool.tile([128, GC * WIN * PB], BF16, tag="w1",
                                         name="w1")
                    nc.sync.dma_start(win1[:], xw1_dram[ds(wi, 1), :, :])
                    win1_v = win1[:].rearrange(
                        "p (g w b) -> p w g b", g=GC, w=WIN)
                    st1["h_prev"] = [
                        h1_cur[:, kc * PB:(kc + 1) * PB] for kc in range(KC)]
                    st1["c_prev"] = c1_cur
                    return win1_v

                # peel: L0 window 0 alone, then bulk xw1 for window 0
                win0_v, hwin = l0_window(0)
                for u in range(WIN):
                    _emit_step(nc, ps0_pool, ew_pool, hw_pool, whh0_sb,
                               win0_v[:, u], st0, u, "0")
                _emit_bulk_xw1(nc, bps_pool, bxw_pool, a1_sb, b1_sb, hwin,
                               xw1_dram, 0)

                # main loop: L0 window iw+1 interleaved with L1 window iw
                with tc.For_i(0, NWIN - 1, 1,
                              hint_engines=(mybir.EngineType.PE,)) as iw:
                    win0_v, hwin = l0_window(iw * WIN + WIN)
                    win1_v = l1_window(iw)
                    for u in range(WIN):
                        _emit_step(nc, ps0_pool, ew_pool, hw_pool, whh0_sb,
                                   win0_v[:, u], st0, u, "0")
                        _emit_step(nc, ps1_pool, ew_pool, hw_pool, whh1_sb,
                                   win1_v[:, u], st1, u, "1")
                    _emit_bulk_xw1(nc, bps_pool, bxw_pool, a1_sb, b1_sb, hwin,
                                   xw1_dram, iw + 1)

                # peel: L1 last window
                win1_v = l1_window(NWIN - 1)
                for u in range(WIN):
                    _emit_step(nc, ps1_pool, ew_pool, hw_pool, whh1_sb,
                               win1_v[:, u], st1, u, "1")

            # ---- phase F: out = relu(h1_last @ Wout.T + bout) ----
            with tc.tile_pool(name="f_ps", bufs=2, space="PSUM") as fps_pool, \
                 tc.tile_pool(name="f_o", bufs=1) as fo_pool:
                out_sb = fo_pool.tile([PB, OUT], F32)
                for half in range(2):
                    psF = fps_pool.tile([PB, 512], F32, tag="psF")
                    for kc in range(KC):
                        nc.tensor.matmul(
                            psF[:],
                            h1_cur[:, kc * PB:(kc + 1) * PB],
                            wout_sb[:, kc, half * 512:(half + 1) * 512],
                            start=(kc == 0),
                            stop=(kc == KC - 1),
                        )
                    sl = slice(half * 512, (half + 1) * 512)
                    nc.vector.tensor_tensor(
                        out_sb[:, sl], psF[:], bout_sb[:, sl], OP.add
                    )
                    nc.vector.tensor_scalar_max(out_sb[:, sl], out_sb[:, sl], 0.0)
                nc.sync.dma_start(out_ext[:, :], out_sb[:])

    _split_drain_waits(nc)
    return nc


_NC_CACHE = None


def _get_nc():
    global _NC_CACHE
    if _NC_CACHE is None:
        _NC_CACHE = build_nc()
    return _NC_CACHE


def _prep_host(inputs):
    x = np.asarray(inputs["x"], dtype=np.float32)
    coef = 1.0 / math.gamma(0.5)
    t = np.arange(T, dtype=np.float64)
    diff = t[:, None] - t[None, :]
    W = np.where(diff > 0, (np.abs(diff) + 1e-6) ** -0.5, 0.0).astype(np.float32)
    d = (coef * W.sum(1)).astype(np.float32)
    G = (np.diag(d) - coef * W).astype(np.float32)  # feats_b = G @ x_b
    GT = np.ascontiguousarray(G.T).reshape(KC, 128, T)

    perm = np.concatenate([  # torch gate order i,f,g,o -> [i,f,o,g]
        np.arange(0, H), np.arange(H, 2 * H),
        np.arange(3 * H, 4 * H), np.arange(2 * H, 3 * H),
    ])
    bf = ml_dtypes.bfloat16

    A0 = np.zeros((G4, NP), np.float32)
    A0[:, :N] = np.asarray(inputs["Wih0"], np.float32)[perm, :N]
    A0T = np.ascontiguousarray(A0.T).astype(bf).reshape(NC2, 128, G4)
    b0 = (np.asarray(inputs["bih0"], np.float32)
          + np.asarray(inputs["bhh0"], np.float32))[perm]
    b0_t = np.ascontiguousarray(b0.reshape(GC, 128).T)
    Whh0T = np.ascontiguousarray(
        np.asarray(inputs["Whh0"], np.float32)[perm].T
    ).astype(bf).reshape(KC, 128, G4)

    A1T = np.ascontiguousarray(
        np.asarray(inputs["Wih1"], np.float32)[perm].T
    ).astype(bf).reshape(KC, 128, G4)
    b1 = (np.asarray(inputs["bih1"], np.float32)
          + np.asarray(inputs["bhh1"], np.float32))[perm]
    b1_t = np.ascontiguousarray(b1.reshape(GC, 128).T)
    Whh1T = np.ascontiguousarray(
        np.asarray(inputs["Whh1"], np.float32)[perm].T
    ).astype(bf).reshape(KC, 128, G4)

    WoutT = np.ascontiguousarray(
        np.asarray(inputs["Wout"], np.float32).T
    ).astype(bf).reshape(KC, 128, OUT)
    bout_r = np.broadcast_to(
        np.asarray(inputs["bout"], np.float32), (PB, OUT)
    ).copy()

    xp = np.zeros((B, T, NP), np.float32)
    xp[:, :, :N] = x

    shared = dict(
        gt=GT, a0t=A0T, b0=b0_t, whh0t=Whh0T, a1t=A1T, b1=b1_t,
        whh1t=Whh1T, woutt=WoutT, boutr=bout_r,
    )
    in_maps = []
    for c in range(NCORES):
        m = dict(shared)
        m["x"] = np.ascontiguousarray(xp[c * PB:(c + 1) * PB])
        in_maps.append(m)
    return in_maps


def kernel(**inputs):
    nc = _get_nc()
    in_maps = _prep_host(inputs)
    res = run_bass_kernel_spmd(nc, in_maps, core_ids=list(range(NCORES)))
    out = np.concatenate([r["out"] for r in res.results], axis=0)
    return out.astype(np.float32)
